# revision 1
# baseline (speedup 1.0000x reference)
"""Multi-head self-attention with RoPE on 8 TRN2 NeuronCores.

Sharding: core c = (b, hg): b = c // 4 (data parallel over batch),
hg = c % 4 (tensor parallel over head groups of 4 heads = 512 features).
Each core computes q/k/v projections for its 4 heads, RoPE, causal
attention, and a partial out-projection [S, E]; the host sums the 4
partials per batch and adds bo.

Device program (per core): two passes over head pairs; within a pass,
s-blocks stream through an interleaved pipeline: project q/k/v for
s-block sb (+fused bias/RoPE on DVE), then attention q-tile qi=sb for
both heads (causality means only k/v chunks <= sb are needed; they are
resident in persistent SBUF tiles, so there are no DRAM spills). The
attention inner loop is software-pipelined so the PE computes
scores(ki+1..ki+2) while ACT exponentiates scores(ki); diagonal k-chunks
compute only the unmasked q-range. The out-projection runs as a final
phase (gt-outer, small wo blocks) that overlaps the pass-1 tail. All
matmuls run in float32r (~1e-4 rel err; full fp32 is 4x slower on PE).

Layouts are chosen so no on-device transposes are needed: the host
feeds xT = x[b].T, W{q,k,v}T pre-transposed (q/k with rope pairs
permuted evens-first so RoPE becomes two 64-partition block multiplies),
and WoT; attention outputs accumulate as [d, q] which is exactly the
lhsT the out-projection needs to produce O[s, g] directly.
"""

import sys

if "/opt/trn_rl_repo" not in sys.path:
    sys.path.insert(0, "/opt/trn_rl_repo")

import numpy as np

import concourse.bass as bass  # noqa: F401  (engine types referenced via nc)
import concourse.mybir as mybir
from concourse import bacc
from concourse.tile import TileContext
from concourse import bass_isa
from concourse.bass_utils import run_bass_kernel_spmd

B, S, E, H, D = 2, 2048, 2048, 16, 128
NCORES = 8
GROUPS = 4          # head groups (tensor parallel)
HPC = H // GROUPS   # heads per core
FH = HPC * D        # features per core (512)
ECH = E // 128      # contraction chunks (16)
SB = 512            # phase-1 s-block width
QT = 512            # attention q-tile width
NQ = S // QT        # 4 q tiles
NST = S // 128      # 16 s chunks
NSB = S // SB       # 4 s-blocks
HPP = 2             # heads per pass
FP = HPP * D        # 256 features per pass

dt = mybir.dt
F32 = dt.float32
F32R = dt.float32r
AX = mybir.AluOpType
ACTF = mybir.ActivationFunctionType

_CACHE = {}


def _build_program():
    nc = bacc.Bacc("TRN2", target_bir_lowering=False, debug=False,
                   num_devices=NCORES)

    xT = nc.dram_tensor("xT", [E, S], F32R, kind="ExternalInput")
    # weights are stored per pass: [pass, E, FP] etc.
    wqT = nc.dram_tensor("wqT", [E, FH], F32R, kind="ExternalInput")
    wkT = nc.dram_tensor("wkT", [E, FH], F32R, kind="ExternalInput")
    wvT = nc.dram_tensor("wvT", [E, FH], F32R, kind="ExternalInput")
    woT = nc.dram_tensor("woT", [FH, E], F32R, kind="ExternalInput")
    bqk = nc.dram_tensor("bqk", [128, 4 * HPC], F32, kind="ExternalInput")
    bv_rep = nc.dram_tensor("bv_rep", [128, FH], F32, kind="ExternalInput")
    cos_d = nc.dram_tensor("cos_t", [128, S], F32, kind="ExternalInput")
    sin_d = nc.dram_tensor("sin_t", [128, S], F32, kind="ExternalInput")
    cmask_d = nc.dram_tensor("cmask", [128, 4 * SB], F32R,
                             kind="ExternalInput")
    out_d = nc.dram_tensor("out", [S, E], F32, kind="ExternalOutput")

    inv_sqrt_d = float(1.0 / np.sqrt(D))

    with TileContext(nc) as tc:
        with (
            tc.tile_pool(name="psum", bufs=2, space="PSUM") as psp,
            tc.tile_pool(name="cst", bufs=1) as cst,
            tc.tile_pool(name="ao0p", bufs=1) as ao0p,
        ):
            bqk_t = cst.tile([128, 4 * HPC], F32, tag="bqk")
            bv_t = cst.tile([128, FH], F32, tag="bv")
            cm_t = cst.tile([128, 4 * SB], F32R, tag="cm")
            nc.sync.dma_start(out=bqk_t[:], in_=bqk[:])
            nc.sync.dma_start(out=bv_t[:], in_=bv_rep[:])
            nc.sync.dma_start(out=cm_t[:], in_=cmask_d[:])

            # attention outputs for all 4 heads stay resident for phase 3
            ao_t = [ao0p.tile([128, S], F32R, tag=f"ao_{h}", name=f"ao_{h}")
                    for h in range(HPC)]

            # PE warm-up: dummy matmuls on the mask tile fill the initial
            # DMA-fill window and lift the HAM clock gate to 2.4 GHz before
            # the first real projection chains run. Result is never read.
            pwarm = psp.tile([128, SB], F32, tag="po", bufs=2, name="pwarm")
            for i in range(16):
                nc.tensor.matmul(pwarm[:], cm_t[:, 0:128], cm_t[:, 0:SB],
                                 start=(i == 0), stop=(i == 15))

            for p in range(2):      # head-pair passes
                fsl = slice(p * FP, (p + 1) * FP)   # feature cols of this pass
                with (
                    tc.tile_pool(name=f"wp{p}", bufs=1) as wp,
                    tc.tile_pool(name=f"kv{p}", bufs=1) as kvp,
                    tc.tile_pool(name=f"xp{p}", bufs=20) as xp,
                    tc.tile_pool(name=f"st{p}", bufs=2) as st1,
                    tc.tile_pool(name=f"cs{p}", bufs=1) as csp,
                ):
                    wq_t = [wp.tile([128, FP], F32R, tag=f"wq{e}",
                                    name=f"wq{p}_{e}") for e in range(ECH)]
                    wk_t = [wp.tile([128, FP], F32R, tag=f"wk{e}",
                                    name=f"wk{p}_{e}") for e in range(ECH)]
                    wv_t = [wp.tile([128, FP], F32R, tag=f"wv{e}",
                                    name=f"wv{p}_{e}") for e in range(ECH)]
                    xs0 = []
                    for e in range(ECH):
                        esl = slice(e * 128, (e + 1) * 128)
                        xt = xp.tile([128, SB], F32R, tag="xslab",
                                     name="xs0")
                        nc.sync.dma_start(out=xt[:], in_=xT[esl, 0:SB])
                        xs0.append(xt)
                        nc.sync.dma_start(out=wq_t[e][:], in_=wqT[esl, fsl])
                        nc.sync.dma_start(out=wk_t[e][:], in_=wkT[esl, fsl])
                        nc.sync.dma_start(out=wv_t[e][:], in_=wvT[esl, fsl])

                    # persistent k/v for this pass's 2 heads
                    kh = [kvp.tile([128, S], F32R, tag=f"kh{h}",
                                   name=f"kh{p}_{h}") for h in range(HPP)]
                    vh = [kvp.tile([128, S], F32R, tag=f"vh{h}",
                                   name=f"vh{p}_{h}") for h in range(HPP)]

                    def emit_loads(sb):
                        ssl = slice(sb * SB, (sb + 1) * SB)
                        if sb == 0:
                            xs = xs0
                        else:
                            xs = []
                            for e in range(ECH):
                                xt = xp.tile([128, SB], F32R, tag="xslab",
                                             name="xs")
                                nc.sync.dma_start(
                                    out=xt[:],
                                    in_=xT[e * 128:(e + 1) * 128, ssl])
                                xs.append(xt)
                        cos_s = csp.tile([128, SB], F32, tag="cos",
                                         name="cos_s", bufs=2)
                        sin_s = csp.tile([128, SB], F32, tag="sin",
                                         name="sin_s", bufs=2)
                        nc.sync.dma_start(out=cos_s[:], in_=cos_d[:, ssl])
                        nc.sync.dma_start(out=sin_s[:], in_=sin_d[:, ssl])
                        return xs, cos_s, sin_s

                    def emit_qk(sb, ft, res):
                        """Project+rope q and k f-tile ft of s-block sb.
                        Returns the roped q tile."""
                        xs, cos_s, sin_s = res
                        ssl = slice(sb * SB, (sb + 1) * SB)
                        hglob = p * HPP + ft
                        qtile = None
                        for kind, wt, bofs in (("q", wq_t, 0),
                                               ("k", wk_t, 2 * HPC)):
                            ps = psp.tile([128, SB], F32, tag="ps1",
                                          bufs=3, name="psqk")
                            for e in range(ECH):
                                nc.tensor.matmul(
                                    ps[:],
                                    wt[e][:, ft * 128:(ft + 1) * 128],
                                    xs[e][:],
                                    start=(e == 0), stop=(e == ECH - 1))
                            bias = bqk_t[:, bofs + hglob:bofs + hglob + 1]
                            bias_sw = bqk_t[:, bofs + HPC + hglob:
                                            bofs + HPC + hglob + 1]
                            qsw = st1.tile([128, SB], F32, tag="qsw", bufs=2)
                            nc.scalar.copy(qsw[0:64, :], ps[64:128, :])
                            nc.scalar.copy(qsw[64:128, :], ps[0:64, :])
                            t1 = st1.tile([128, SB], F32R, tag="t1", bufs=2)
                            nc.vector.scalar_tensor_tensor(
                                out=t1[:], in0=ps[:], scalar=bias,
                                in1=cos_s[:], op0=AX.add, op1=AX.mult)
                            if kind == "q":
                                dst = st1.tile([128, SB], F32R, tag="qh",
                                               bufs=4, name="qh")
                                qtile = dst
                                dview = dst[:]
                            else:
                                dview = kh[ft][:, ssl]
                            t2 = st1.tile([128, SB], F32R, tag="t2", bufs=2)
                            nc.vector.scalar_tensor_tensor(
                                out=t2[:], in0=qsw[:], scalar=bias_sw,
                                in1=sin_s[:], op0=AX.add, op1=AX.mult)
                            nc.gpsimd.tensor_add(dview, t1[:], t2[:])
                        return qtile

                    def emit_v(sb, res):
                        xs, _, _ = res
                        for ssub in range(SB // 128):
                            ps = psp.tile([128, FP], F32, tag="ps1",
                                          bufs=3, name="psv")
                            for e in range(ECH):
                                nc.tensor.matmul(
                                    ps[:],
                                    xs[e][:, ssub * 128:(ssub + 1) * 128],
                                    wv_t[e][:],
                                    start=(e == 0), stop=(e == ECH - 1))
                            scol = sb * SB + ssub * 128
                            for ft in range(HPP):
                                hglob = p * HPP + ft
                                nc.vector.tensor_add(
                                    vh[ft][:, scol:scol + 128],
                                    ps[:, ft * 128:(ft + 1) * 128],
                                    bv_t[:, hglob * 128:(hglob + 1) * 128])


                    def emit_attn(sb, ft, qtile):
                        """Causal attention q-tile sb for head ft
                        (software-pipelined over k-chunks)."""
                        ssl = slice(sb * SB, (sb + 1) * SB)
                        nk = (sb + 1) * (SB // 128)
                        po = psp.tile([128, SB], F32, tag="po", bufs=2,
                                      name="po")
                        # softmax denominator: running adds on DVE (frees
                        # both the ones-matmuls on the PE and a PSUM bank)
                        dacc = st1.tile([128, SB], F32, tag="dacc", bufs=2)
                        pending = []
                        for ki in range(nk):
                            j = ki - sb * (SB // 128)
                            q0 = 128 * j if j > 0 else 0
                            ksl = slice(ki * 128, (ki + 1) * 128)
                            pscore = psp.tile([128, SB], F32, tag="pscore",
                                              bufs=3, name="pscore")
                            nc.tensor.matmul(
                                pscore[:, q0:SB], kh[ft][:, ksl],
                                qtile[:, q0:SB], start=True, stop=True)
                            pexp = st1.tile([128, SB], F32R, tag="pexp",
                                            bufs=4)
                            nc.scalar.activation(
                                pexp[:, q0:SB], pscore[:, q0:SB], ACTF.Exp,
                                scale=inv_sqrt_d)
                            if j >= 0:
                                nc.vector.tensor_mul(
                                    pexp[:, q0:SB], pexp[:, q0:SB],
                                    cm_t[:, j * SB + q0:(j + 1) * SB])
                            if ki == 0:
                                nc.scalar.copy(dacc[:], pexp[:])
                            else:
                                nc.vector.tensor_add(
                                    dacc[:, q0:SB], dacc[:, q0:SB],
                                    pexp[:, q0:SB])
                            pending.append((ki, pexp, q0))
                            if len(pending) > 2:
                                k0, px, pq0 = pending.pop(0)
                                k0sl = slice(k0 * 128, (k0 + 1) * 128)
                                nc.tensor.matmul(
                                    po[:, pq0:SB], vh[ft][:, k0sl],
                                    px[:, pq0:SB],
                                    start=(k0 == 0), stop=False)
                        while pending:
                            k0, px, pq0 = pending.pop(0)
                            last = not pending
                            k0sl = slice(k0 * 128, (k0 + 1) * 128)
                            nc.tensor.matmul(po[:, pq0:SB], vh[ft][:, k0sl],
                                             px[:, pq0:SB],
                                             start=(k0 == 0), stop=last)
                        # cross-partition sum on the (idle) GpSimd engine
                        dred = st1.tile([128, SB], F32, tag="dred", bufs=2)
                        nc.gpsimd.partition_all_reduce(
                            out_ap=dred[:], in_ap=dacc[:], channels=128,
                            reduce_op=bass_isa.ReduceOp.add)
                        rec = st1.tile([128, SB], F32, tag="pexp", bufs=4,
                                       name="rec")
                        nc.vector.reciprocal(rec[:], dred[:])
                        nc.vector.tensor_mul(ao_t[p * HPP + ft][:, ssl],
                                             po[:], rec[:])

                    # stage-level software pipeline: next stage's projection
                    # chains are emitted between this stage's two attention
                    # heads so the PE always has independent work while the
                    # po bank turns around.
                    res = emit_loads(0)
                    q_cur = [emit_qk(0, 0, res), emit_qk(0, 1, res)]
                    emit_v(0, res)
                    for sb in range(NSB):
                        nxt = sb + 1
                        if nxt < NSB:
                            res_n = emit_loads(nxt)
                        emit_attn(sb, 0, q_cur[0])
                        if nxt < NSB:
                            q_next0 = emit_qk(nxt, 0, res_n)
                        emit_attn(sb, 1, q_cur[1])
                        if nxt < NSB:
                            q_next1 = emit_qk(nxt, 1, res_n)
                            emit_v(nxt, res_n)
                            q_cur = [q_next0, q_next1]

            # ---------------- Phase 3: out projection (partial) ----------
            # gt-outer with small per-column wo blocks so the first column
            # block can start while pass-1 attention is still finishing.
            with (
                tc.tile_pool(name="wop", bufs=1) as wop,
                tc.tile_pool(name="op3", bufs=4) as op3,
            ):
                for gt in range(E // 512):
                    gsl = slice(gt * 512, (gt + 1) * 512)
                    wob = [wop.tile([128, 512], F32R, tag=f"wob{f}",
                                    name=f"wob{gt}_{f}", bufs=2)
                           for f in range(HPC)]
                    for f in range(HPC):
                        nc.sync.dma_start(
                            out=wob[f][:],
                            in_=woT[f * 128:(f + 1) * 128, gsl])
                    for st in range(S // 128):
                        stsl = slice(st * 128, (st + 1) * 128)
                        psO = psp.tile([128, 512], F32, tag="ps1", bufs=3,
                                       name="psO")
                        for f in range(HPC):
                            nc.tensor.matmul(psO[:], ao_t[f][:, stsl],
                                             wob[f][:],
                                             start=(f == 0),
                                             stop=(f == HPC - 1))
                        osb = op3.tile([128, 512], F32, tag="osb", bufs=6)
                        if st % 2 == 0:
                            nc.vector.tensor_copy(osb[:], psO[:])
                        else:
                            nc.scalar.copy(osb[:], psO[:])
                        nc.sync.dma_start(out=out_d[stsl, gsl], in_=osb[:])

    nc.compile()
    return nc


def _host_constants():
    """RoPE cos/sin tables (evens-first layout) and causal masks."""
    i = np.arange(64, dtype=np.float64)
    freqs = np.power(10000.0, -2.0 * i / D)          # theta per rope pair
    pos = np.arange(S, dtype=np.float64)
    ang = pos[None, :] * freqs[:, None]              # [64, S]
    cos = np.cos(ang).astype(np.float32)
    sin = np.sin(ang).astype(np.float32)
    cos_t = np.concatenate([cos, cos], axis=0)       # [128, S]
    sin_t = np.concatenate([-sin, sin], axis=0)      # [128, S] signed
    r = np.arange(128)[:, None]
    c = np.arange(QT)[None, :]
    masks = [(128 * j + r <= c).astype(np.float32) for j in range(QT // 128)]
    cmask = np.concatenate(masks, axis=1)            # [128, 4*QT]
    return cos_t, sin_t, cmask


def kernel(x, Wq, bq, Wk, bk, Wv, bv, Wo, bo):
    x = np.asarray(x, dtype=np.float32)
    Wq = np.asarray(Wq, dtype=np.float32)
    bq = np.asarray(bq, dtype=np.float32)
    Wk = np.asarray(Wk, dtype=np.float32)
    bk = np.asarray(bk, dtype=np.float32)
    Wv = np.asarray(Wv, dtype=np.float32)
    bv = np.asarray(bv, dtype=np.float32)
    Wo = np.asarray(Wo, dtype=np.float32)
    bo = np.asarray(bo, dtype=np.float32)

    if "nc" not in _CACHE:
        _CACHE["nc"] = _build_program()
        _CACHE["consts"] = _host_constants()
    nc = _CACHE["nc"]
    cos_t, sin_t, cmask = _CACHE["consts"]

    # evens-first permutation of each head's 128 dims
    perm = np.concatenate([np.arange(0, D, 2), np.arange(1, D, 2)])

    xT = [np.ascontiguousarray(x[b].T) for b in range(B)]

    in_maps = []
    for c in range(NCORES):
        b, hg = divmod(c, GROUPS)
        rows = slice(hg * FH, (hg + 1) * FH)
        Wq_s = Wq[rows].reshape(HPC, D, E)[:, perm, :].reshape(FH, E)
        Wk_s = Wk[rows].reshape(HPC, D, E)[:, perm, :].reshape(FH, E)
        bq_s = bq[rows].reshape(HPC, D)[:, perm]     # [HPC, 128]
        bk_s = bk[rows].reshape(HPC, D)[:, perm]
        sw = np.concatenate([np.arange(64, 128), np.arange(0, 64)])
        bqk_t = np.concatenate(
            [bq_s, bq_s[:, sw], bk_s, bk_s[:, sw]], axis=0).T.astype(np.float32)
        bqk_t = np.ascontiguousarray(bqk_t)          # [128, 4*HPC]
        in_maps.append({
            "xT": xT[b],
            "wqT": np.ascontiguousarray(Wq_s.T),
            "wkT": np.ascontiguousarray(Wk_s.T),
            "wvT": np.ascontiguousarray(Wv[rows].T),
            "woT": np.ascontiguousarray(Wo[:, rows].T),
            "bqk": bqk_t,
            "bv_rep": np.ascontiguousarray(
                np.broadcast_to(bv[rows], (128, FH))),
            "cos_t": cos_t,
            "sin_t": sin_t,
            "cmask": cmask,
        })

    res = run_bass_kernel_spmd(nc, in_maps, list(range(NCORES)))
    outs = [res.results[c]["out"] for c in range(NCORES)]

    result = np.empty((B, S, E), dtype=np.float32)
    for b in range(B):
        acc = outs[GROUPS * b].astype(np.float32)
        for g in range(1, GROUPS):
            acc = acc + outs[GROUPS * b + g]
        result[b] = acc + bo[None, :]
    return result



# revision 2
# speedup vs baseline: 1.1718x; 1.1718x over previous
"""Multi-head self-attention with RoPE on 8 TRN2 NeuronCores.

Sharding: core c = (b, hg): b = c // 4 (data parallel over batch),
hg = c % 4 (tensor parallel over head groups of 4 heads = 512 features).
Each core computes q/k/v projections for its 4 heads, RoPE, causal
attention, and a partial out-projection [S, E] in bf16; the host sums
the 4 partials per batch and adds bo.

Performance scheme (single pass over all 4 heads):
- q/k/v and out projections run as fp8e4m3 DoubleRow matmuls (0.5
  cycles/row, 2x128 contraction per instruction) with a 3-product
  hi/lo residual split (x_hi@W_hi + x_lo@W_hi + x_hi@W_lo) that keeps
  quantization error at the ~1e-3 level: 0.75 cycles per fp32r-row
  equivalent. x and W splits are precomputed on the host; the
  attention-output split is computed on device (ACT cast + DVE sub).
- Attention (scores, attn@V) runs in bf16 (1 cycle/row, exact f32
  accumulation in PSUM). RoPE runs as 3 DVE scalar_tensor_tensor ops
  per tensor (full-width cos term + two half-partition swapped sin
  terms) with the 1/1024 projection descale folded into the bf16
  cos/sin tables, combining on the Pool engine.
- The softmax denominator accumulates in f32 from bf16 chunk-pair sums
  (DVE), is partition-reduced on Pool, and reciprocal'd on DVE. Causal
  masks multiply on Pool. exp runs on ACT writing bf16.
- Diagonal k-chunks compute only from the covering pair start so
  chunk-pair ops (dacc) see fully-masked zeros in the extension.
"""

import sys

if "/opt/trn_rl_repo" not in sys.path:
    sys.path.insert(0, "/opt/trn_rl_repo")

import numpy as np
import ml_dtypes

import concourse.bass as bass  # noqa: F401
import concourse.mybir as mybir
from concourse import bacc
from concourse.tile import TileContext
from concourse.bass_utils import run_bass_kernel_spmd

B, S, E, H, D = 2, 2048, 2048, 16, 128
NCORES = 8
GROUPS = 4          # head groups (tensor parallel)
HPC = H // GROUPS   # heads per core (4)
FH = HPC * D        # features per core (512)
ECH = E // 128      # contraction chunks (16)
NPAIR = ECH // 2    # DoubleRow chunk pairs (8)
SB = 512            # s-block width
NSB = S // SB       # 4 s-blocks
NST = S // 128      # 16 s chunks

SX = 16.0           # fp8 scale on x
SW = 64.0           # fp8 scale on weights
SAO = 16.0          # fp8 scale on attention output
PRJ = SX * SW       # projection psum scale (1024)

dt = mybir.dt
F32 = dt.float32
BF16 = dt.bfloat16
F8 = dt.float8e4
AX = mybir.AluOpType
ACTF = mybir.ActivationFunctionType
DR = mybir.MatmulPerfMode.DoubleRow
F8NP = ml_dtypes.float8_e4m3
BFNP = ml_dtypes.bfloat16

_CACHE = {}


def _build_program():
    nc = bacc.Bacc("TRN2", target_bir_lowering=False, debug=False,
                   num_devices=NCORES)

    xhi_d = nc.dram_tensor("xhi", [128, ECH, S], F8, kind="ExternalInput")
    xlo_d = nc.dram_tensor("xlo", [128, ECH, S], F8, kind="ExternalInput")
    w_d = {}
    for nm in ("wq", "wk", "wv"):
        for hl in ("hi", "lo"):
            w_d[nm + hl] = nc.dram_tensor(nm + hl, [128, ECH, FH], F8,
                                          kind="ExternalInput")
    wohi_d = nc.dram_tensor("wohi", [128, HPC, E], F8, kind="ExternalInput")
    wolo_d = nc.dram_tensor("wolo", [128, HPC, E], F8, kind="ExternalInput")
    bqk_d = nc.dram_tensor("bqk", [128, 4 * HPC], F32, kind="ExternalInput")
    bv_d = nc.dram_tensor("bv_rep", [128, FH], BF16, kind="ExternalInput")
    cos_d = nc.dram_tensor("cos_t", [128, S], BF16, kind="ExternalInput")
    sin_d = nc.dram_tensor("sin_t", [128, S], BF16, kind="ExternalInput")
    cmask_d = nc.dram_tensor("cmask", [128, 4 * SB], BF16,
                             kind="ExternalInput")
    out_d = nc.dram_tensor("out", [S, E], BF16, kind="ExternalOutput")

    inv_sqrt_d = float(1.0 / np.sqrt(D))

    with TileContext(nc) as tc:
        with (
            tc.tile_pool(name="psum", bufs=2, space="PSUM") as psp,
            tc.tile_pool(name="cst", bufs=1) as cst,
            tc.tile_pool(name="wp", bufs=1) as wp,
            tc.tile_pool(name="kv", bufs=1) as kvp,
            tc.tile_pool(name="xp", bufs=2) as xp,
            tc.tile_pool(name="st", bufs=2) as st1,
            tc.tile_pool(name="aop", bufs=2) as aop,
            tc.tile_pool(name="osp", bufs=3) as osp,
        ):
            cm_t = cst.tile([128, 4 * SB], BF16, tag="cm")
            bqk_t = cst.tile([128, 4 * HPC], F32, tag="bqk")
            bv_t = cst.tile([128, FH], BF16, tag="bv")
            cos_t = cst.tile([128, S], BF16, tag="cos")
            sin_t = cst.tile([128, S], BF16, tag="sin")
            nc.sync.dma_start(out=cm_t[:], in_=cmask_d[:])
            nc.sync.dma_start(out=bqk_t[:], in_=bqk_d[:])
            nc.sync.dma_start(out=bv_t[:], in_=bv_d[:])

            def load_x(sb):
                ssl = slice(sb * SB, (sb + 1) * SB)
                xh = xp.tile([128, ECH, SB], F8, tag="xh", name="xh")
                xl = xp.tile([128, ECH, SB], F8, tag="xl", name="xl")
                nc.sync.dma_start(out=xh[:], in_=xhi_d[:, :, ssl])
                nc.sync.dma_start(out=xl[:], in_=xlo_d[:, :, ssl])
                return xh, xl

            xs0 = load_x(0)

            wt = {}
            for nm in ("wq", "wk", "wv"):
                for hl in ("hi", "lo"):
                    t = wp.tile([128, ECH, FH], F8, tag=nm + hl)
                    nc.sync.dma_start(out=t[:], in_=w_d[nm + hl][:])
                    wt[nm + hl] = t
            nc.sync.dma_start(out=cos_t[:], in_=cos_d[:])
            nc.sync.dma_start(out=sin_t[:], in_=sin_d[:])
            wo_hi = wp.tile([128, HPC, E], F8, tag="wohi")
            wo_lo = wp.tile([128, HPC, E], F8, tag="wolo")
            nc.sync.dma_start(out=wo_hi[:], in_=wohi_d[:])
            nc.sync.dma_start(out=wo_lo[:], in_=wolo_d[:])

            # persistent k (per head, [d, S]) and v ([s128, (chunk, h, d)])
            kh = [kvp.tile([128, S], BF16, tag=f"kh{h}", name=f"kh{h}")
                  for h in range(HPC)]
            vh = kvp.tile([128, NST, FH], BF16, tag="vh")

            # PE warm-up: lifts the clock gate while initial DMAs fill.
            pwarm = psp.tile([128, SB], F32, tag="po", bufs=2, name="pwarm")
            for i in range(24):
                nc.tensor.matmul(pwarm[:], cm_t[:, 0:128], cm_t[:, 0:SB],
                                 start=(i == 0), stop=(i == 23))

            def emit_qk(sb, h, xs):
                """Project+rope q and k for head h of s-block sb."""
                xh, xl = xs
                ssl = slice(sb * SB, (sb + 1) * SB)
                fsl = slice(h * 128, (h + 1) * 128)
                qtile = None
                for kind in ("q", "k"):
                    whi = wt[("wq" if kind == "q" else "wk") + "hi"]
                    wlo = wt[("wq" if kind == "q" else "wk") + "lo"]
                    ps = psp.tile([128, SB], F32, tag="ps1", bufs=3,
                                  name="psqk")
                    for j in range(NPAIR):
                        jp = slice(2 * j, 2 * j + 2)
                        nc.tensor.matmul(ps[:], whi[:, jp, fsl], xh[:, jp, :],
                                         start=(j == 0), stop=False,
                                         perf_mode=DR)
                    for j in range(NPAIR):
                        jp = slice(2 * j, 2 * j + 2)
                        nc.tensor.matmul(ps[:], whi[:, jp, fsl], xl[:, jp, :],
                                         start=False, stop=False,
                                         perf_mode=DR)
                    for j in range(NPAIR):
                        jp = slice(2 * j, 2 * j + 2)
                        nc.tensor.matmul(ps[:], wlo[:, jp, fsl], xh[:, jp, :],
                                         start=False, stop=(j == NPAIR - 1),
                                         perf_mode=DR)
                    bofs = 0 if kind == "q" else 2 * HPC
                    bias = bqk_t[:, bofs + h:bofs + h + 1]
                    bias_sw = bqk_t[:, bofs + HPC + h:bofs + HPC + h + 1]
                    t1 = st1.tile([128, SB], BF16, tag="t1", bufs=2)
                    nc.vector.scalar_tensor_tensor(
                        out=t1[:], in0=ps[:], scalar=bias,
                        in1=cos_t[:, ssl], op0=AX.add, op1=AX.mult)
                    t2 = st1.tile([128, SB], BF16, tag="t2", bufs=2)
                    nc.vector.scalar_tensor_tensor(
                        out=t2[0:64, :], in0=ps[64:128, :],
                        scalar=bias_sw[0:64], in1=sin_t[0:64, ssl],
                        op0=AX.add, op1=AX.mult)
                    nc.vector.scalar_tensor_tensor(
                        out=t2[64:128, :], in0=ps[0:64, :],
                        scalar=bias_sw[64:128], in1=sin_t[64:128, ssl],
                        op0=AX.add, op1=AX.mult)
                    if kind == "q":
                        dst = st1.tile([128, SB], BF16, tag="qh", bufs=8,
                                       name="qh")
                        qtile = dst
                        dview = dst[:]
                    else:
                        dview = kh[h][:, ssl]
                    nc.gpsimd.tensor_add(dview, t1[:], t2[:])
                return qtile

            def emit_v(sb, xs):
                xh, xl = xs
                for ssub in range(SB // 128):
                    scol = slice(ssub * 128, (ssub + 1) * 128)
                    ps = psp.tile([128, FH], F32, tag="ps1", bufs=3,
                                  name="psv")
                    for j in range(NPAIR):
                        jp = slice(2 * j, 2 * j + 2)
                        nc.tensor.matmul(ps[:], xh[:, jp, scol],
                                         wt["wvhi"][:, jp, :],
                                         start=(j == 0), stop=False,
                                         perf_mode=DR)
                    for j in range(NPAIR):
                        jp = slice(2 * j, 2 * j + 2)
                        nc.tensor.matmul(ps[:], xl[:, jp, scol],
                                         wt["wvhi"][:, jp, :],
                                         start=False, stop=False,
                                         perf_mode=DR)
                    for j in range(NPAIR):
                        jp = slice(2 * j, 2 * j + 2)
                        nc.tensor.matmul(ps[:], xh[:, jp, scol],
                                         wt["wvlo"][:, jp, :],
                                         start=False, stop=(j == NPAIR - 1),
                                         perf_mode=DR)
                    # vh = ps/PRJ + bv  (bf16)
                    nc.vector.scalar_tensor_tensor(
                        out=vh[:, sb * 4 + ssub, :], in0=ps[:],
                        scalar=float(1.0 / PRJ), in1=bv_t[:],
                        op0=AX.mult, op1=AX.add)

            def emit_attn(sb, h, qtile, aohi, aolo):
                """Causal attention q-tile sb for head h (bf16)."""
                nk = (sb + 1) * 4
                po = psp.tile([128, SB], F32, tag="po", bufs=2, name="po")
                dacc = st1.tile([128, SB], F32, tag="dacc", bufs=2)
                pexps = []      # (ki, pexp, q0)
                pending = []
                for ki in range(nk):
                    j = ki - sb * 4
                    # diagonal chunks start at their pair's q0 so pair ops
                    # see fully-masked zeros in the extension
                    q0 = 128 * (j - (j % 2)) if j > 0 else 0
                    ksl = slice(ki * 128, (ki + 1) * 128)
                    pscore = psp.tile([128, SB], F32, tag="pscore",
                                      bufs=3, name="pscore")
                    nc.tensor.matmul(pscore[:, q0:SB], kh[h][:, ksl],
                                     qtile[:, q0:SB], start=True, stop=True)
                    pexp = st1.tile([128, SB], BF16, tag="pexp", bufs=6)
                    nc.scalar.activation(pexp[:, q0:SB], pscore[:, q0:SB],
                                         ACTF.Exp, scale=inv_sqrt_d)
                    if j >= 0:
                        nc.gpsimd.tensor_mul(
                            pexp[:, q0:SB], pexp[:, q0:SB],
                            cm_t[:, j * SB + q0:(j + 1) * SB])
                    pexps.append((ki, pexp, q0))
                    if ki % 2 == 1:
                        _, pa, pq0 = pexps[ki - 1]
                        if ki == 1:
                            nc.vector.tensor_add(dacc[:, pq0:SB],
                                                 pa[:, pq0:SB],
                                                 pexp[:, pq0:SB])
                        else:
                            sp = st1.tile([128, SB], BF16, tag="spair",
                                          bufs=2)
                            nc.vector.tensor_add(sp[:, pq0:SB],
                                                 pa[:, pq0:SB],
                                                 pexp[:, pq0:SB])
                            nc.vector.tensor_add(dacc[:, pq0:SB],
                                                 dacc[:, pq0:SB],
                                                 sp[:, pq0:SB])
                    pending.append((ki, pexp, q0))
                    if len(pending) > 2:
                        k0, px, pq = pending.pop(0)
                        nc.tensor.matmul(
                            po[:, pq:SB],
                            vh[:, k0, h * 128:(h + 1) * 128],
                            px[:, pq:SB], start=(k0 == 0), stop=False)
                while pending:
                    k0, px, pq = pending.pop(0)
                    last = not pending
                    nc.tensor.matmul(po[:, pq:SB],
                                     vh[:, k0, h * 128:(h + 1) * 128],
                                     px[:, pq:SB], start=(k0 == 0), stop=last)
                dred = st1.tile([128, SB], F32, tag="dred", bufs=2)
                nc.gpsimd.partition_all_reduce(
                    out_ap=dred[:], in_ap=dacc[:], channels=128,
                    reduce_op=__import__("concourse.bass_isa",
                                         fromlist=["ReduceOp"]).ReduceOp.add)
                rec = st1.tile([128, SB], F32, tag="rec", bufs=2)
                nc.vector.reciprocal(rec[:], dred[:])
                t = st1.tile([128, SB], BF16, tag="taot", bufs=2)
                nc.vector.scalar_tensor_tensor(
                    out=t[:], in0=po[:], scalar=SAO, in1=rec[:],
                    op0=AX.mult, op1=AX.mult)
                nc.scalar.copy(aohi[:, h, :], t[:])
                nc.vector.tensor_sub(aolo[:, h, :], t[:], aohi[:, h, :])

            def emit_outproj(sb, aohi, aolo):
                for sti in range(SB // 128):
                    stsl = slice(sti * 128, (sti + 1) * 128)
                    osb = osp.tile([128, E], BF16, tag="osb", name="osb")
                    for gt in range(E // 512):
                        gsl = slice(gt * 512, (gt + 1) * 512)
                        psO = psp.tile([128, 512], F32, tag="ps1", bufs=3,
                                       name="psO")
                        for j in range(HPC // 2):
                            jp = slice(2 * j, 2 * j + 2)
                            nc.tensor.matmul(psO[:], aohi[:, jp, stsl],
                                             wo_hi[:, jp, gsl],
                                             start=(j == 0), stop=False,
                                             perf_mode=DR)
                        for j in range(HPC // 2):
                            jp = slice(2 * j, 2 * j + 2)
                            nc.tensor.matmul(psO[:], aolo[:, jp, stsl],
                                             wo_hi[:, jp, gsl],
                                             start=False, stop=False,
                                             perf_mode=DR)
                        for j in range(HPC // 2):
                            jp = slice(2 * j, 2 * j + 2)
                            nc.tensor.matmul(psO[:], aohi[:, jp, stsl],
                                             wo_lo[:, jp, gsl],
                                             start=False,
                                             stop=(j == HPC // 2 - 1),
                                             perf_mode=DR)
                        nc.scalar.activation(osb[:, gsl], psO[:], ACTF.Copy,
                                             scale=float(1.0 / (SAO * SW)))
                    row0 = sb * SB + sti * 128
                    nc.sync.dma_start(out=out_d[row0:row0 + 128, :],
                                      in_=osb[:])

            # ---- stage loop: attn(sb) interleaved with proj(sb+1) ----
            q_cur = [emit_qk(0, h, xs0) for h in range(HPC)]
            emit_v(0, xs0)
            for sb in range(NSB):
                nxt = sb + 1
                if nxt < NSB:
                    xs_n = load_x(nxt)
                aohi = aop.tile([128, HPC, SB], F8, tag="aohi", name="aohi")
                aolo = aop.tile([128, HPC, SB], F8, tag="aolo", name="aolo")
                q_next = []
                for h in range(HPC):
                    emit_attn(sb, h, q_cur[h], aohi, aolo)
                    if nxt < NSB:
                        q_next.append(emit_qk(nxt, h, xs_n))
                if nxt < NSB:
                    emit_v(nxt, xs_n)
                emit_outproj(sb, aohi, aolo)
                q_cur = q_next

    nc.compile()
    return nc


def _host_constants():
    """RoPE cos/sin tables (evens-first, pre-descaled) and causal masks."""
    i = np.arange(64, dtype=np.float64)
    freqs = np.power(10000.0, -2.0 * i / D)
    pos = np.arange(S, dtype=np.float64)
    ang = pos[None, :] * freqs[:, None]              # [64, S]
    cos = np.cos(ang)
    sin = np.sin(ang)
    cos_t = (np.concatenate([cos, cos], axis=0) / PRJ).astype(BFNP)
    sin_t = (np.concatenate([-sin, sin], axis=0) / PRJ).astype(BFNP)
    r = np.arange(128)[:, None]
    c = np.arange(SB)[None, :]
    masks = [(128 * j + r <= c).astype(np.float32) for j in range(4)]
    cmask = np.concatenate(masks, axis=1).astype(BFNP)
    return cos_t, sin_t, cmask


def _split8(t, s):
    hi = (s * t).astype(F8NP)
    lo = (s * t - hi.astype(np.float32)).astype(F8NP)
    return hi, lo


def _chunked(t, nch):
    """[nch*128, N] f8 -> [128, nch, N]"""
    n = t.shape[1]
    return np.ascontiguousarray(
        t.reshape(nch, 128, n).transpose(1, 0, 2))


def kernel(x, Wq, bq, Wk, bk, Wv, bv, Wo, bo):
    x = np.asarray(x, dtype=np.float32)
    Wq = np.asarray(Wq, dtype=np.float32)
    bq = np.asarray(bq, dtype=np.float32)
    Wk = np.asarray(Wk, dtype=np.float32)
    bk = np.asarray(bk, dtype=np.float32)
    Wv = np.asarray(Wv, dtype=np.float32)
    bv = np.asarray(bv, dtype=np.float32)
    Wo = np.asarray(Wo, dtype=np.float32)
    bo = np.asarray(bo, dtype=np.float32)

    if "nc" not in _CACHE:
        _CACHE["nc"] = _build_program()
        _CACHE["consts"] = _host_constants()
    nc = _CACHE["nc"]
    cos_t, sin_t, cmask = _CACHE["consts"]

    perm = np.concatenate([np.arange(0, D, 2), np.arange(1, D, 2)])
    sw64 = np.concatenate([np.arange(64, 128), np.arange(0, 64)])

    xsplit = []
    for b in range(B):
        xT = np.ascontiguousarray(x[b].T)
        xh, xl = _split8(xT, SX)
        xsplit.append((_chunked(xh, ECH), _chunked(xl, ECH)))

    in_maps = []
    for c in range(NCORES):
        b, hg = divmod(c, GROUPS)
        rows = slice(hg * FH, (hg + 1) * FH)
        Wq_s = Wq[rows].reshape(HPC, D, E)[:, perm, :].reshape(FH, E)
        Wk_s = Wk[rows].reshape(HPC, D, E)[:, perm, :].reshape(FH, E)
        bq_s = bq[rows].reshape(HPC, D)[:, perm]     # [HPC, 128]
        bk_s = bk[rows].reshape(HPC, D)[:, perm]
        bqk_t = PRJ * np.concatenate(
            [bq_s, bq_s[:, sw64], bk_s, bk_s[:, sw64]],
            axis=0).T.astype(np.float32)
        bqk_t = np.ascontiguousarray(bqk_t)          # [128, 4*HPC]

        wqh, wql = _split8(np.ascontiguousarray(Wq_s.T), SW)
        wkh, wkl = _split8(np.ascontiguousarray(Wk_s.T), SW)
        wvh, wvl = _split8(np.ascontiguousarray(Wv[rows].T), SW)
        woh, wol = _split8(np.ascontiguousarray(Wo[:, rows].T), SW)

        in_maps.append({
            "xhi": xsplit[b][0],
            "xlo": xsplit[b][1],
            "wqhi": _chunked(wqh, ECH), "wqlo": _chunked(wql, ECH),
            "wkhi": _chunked(wkh, ECH), "wklo": _chunked(wkl, ECH),
            "wvhi": _chunked(wvh, ECH), "wvlo": _chunked(wvl, ECH),
            "wohi": _chunked(woh, HPC), "wolo": _chunked(wol, HPC),
            "bqk": bqk_t,
            "bv_rep": np.ascontiguousarray(
                np.broadcast_to(bv[rows], (128, FH))).astype(BFNP),
            "cos_t": cos_t,
            "sin_t": sin_t,
            "cmask": cmask,
        })

    res = run_bass_kernel_spmd(nc, in_maps, list(range(NCORES)))
    outs = [res.results[c]["out"] for c in range(NCORES)]

    result = np.empty((B, S, E), dtype=np.float32)
    for b in range(B):
        acc = outs[GROUPS * b].astype(np.float32)
        for g in range(1, GROUPS):
            acc = acc + outs[GROUPS * b + g].astype(np.float32)
        result[b] = acc + bo[None, :]
    return result


# revision 7
# speedup vs baseline: 1.3396x; 1.1432x over previous
"""Multi-head self-attention with RoPE on 8 TRN2 NeuronCores.

Sharding: core c = (b, hg): b = c // 4 (data parallel over batch),
hg = c % 4 (tensor parallel over head groups of 4 heads = 512 features).
Each core computes q/k/v projections for its 4 heads, RoPE, causal
attention, and a partial out-projection [S, E] in bf16; the host sums
the 4 partials per batch and adds bo.

Performance scheme (single pass over all 4 heads):
- q/k/v and out projections run as fp8e4m3 DoubleRow matmuls (0.5
  cycles/row, 2x128 contraction per instruction) with a 3-product
  hi/lo residual split (x_hi@W_hi + x_lo@W_hi + x_hi@W_lo) that keeps
  quantization error at the ~1e-3 level: 0.75 cycles per fp32r-row
  equivalent. x and W splits are precomputed on the host; the
  attention-output split is computed on device (ACT cast + DVE sub).
- Attention (scores, attn@V) runs in bf16 (1 cycle/row, exact f32
  accumulation in PSUM). RoPE runs as 3 DVE scalar_tensor_tensor ops
  per tensor (full-width cos term + two half-partition swapped sin
  terms) with the 1/1024 projection descale folded into the bf16
  cos/sin tables, combining on the Pool engine.
- The softmax denominator accumulates in f32 from bf16 chunk-pair sums
  (DVE), is partition-reduced on Pool, and reciprocal'd on DVE. Causal
  masks multiply on Pool. exp runs on ACT writing bf16.
- Diagonal k-chunks compute only from the covering pair start so
  chunk-pair ops (dacc) see fully-masked zeros in the extension.
"""

import sys

if "/opt/trn_rl_repo" not in sys.path:
    sys.path.insert(0, "/opt/trn_rl_repo")

import numpy as np
import ml_dtypes

import concourse.bass as bass  # noqa: F401
import concourse.mybir as mybir
from concourse import bacc
from concourse.tile import TileContext
from concourse.bass_utils import run_bass_kernel_spmd

B, S, E, H, D = 2, 2048, 2048, 16, 128
NCORES = 8
GROUPS = 4          # head groups (tensor parallel)
HPC = H // GROUPS   # heads per core (4)
FH = HPC * D        # features per core (512)
ECH = E // 128      # contraction chunks (16)
NPAIR = ECH // 2    # DoubleRow chunk pairs (8)
SB = 512            # s-block width
NSB = S // SB       # 4 s-blocks
NST = S // 128      # 16 s chunks

SX = 16.0           # fp8 scale on x
SW = 64.0           # fp8 scale on weights
SAO = 16.0          # fp8 scale on attention output
PRJ = SX * SW       # projection psum scale (1024)

dt = mybir.dt
F32 = dt.float32
BF16 = dt.bfloat16
F8 = dt.float8e4
AX = mybir.AluOpType
ACTF = mybir.ActivationFunctionType
DR = mybir.MatmulPerfMode.DoubleRow
F8NP = ml_dtypes.float8_e4m3
BFNP = ml_dtypes.bfloat16

_CACHE = {}


def _build_program():
    nc = bacc.Bacc("TRN2", target_bir_lowering=False, debug=False,
                   num_devices=NCORES)

    xhi_d = nc.dram_tensor("xhi", [128, ECH, S], F8, kind="ExternalInput")
    xlo_d = nc.dram_tensor("xlo", [128, ECH, S], F8, kind="ExternalInput")
    w_d = {}
    for nm in ("wq", "wk", "wv"):
        for hl in ("hi", "lo"):
            w_d[nm + hl] = nc.dram_tensor(nm + hl, [128, ECH, FH], F8,
                                          kind="ExternalInput")
    wohi_d = nc.dram_tensor("wohi", [128, HPC, E], F8, kind="ExternalInput")
    wolo_d = nc.dram_tensor("wolo", [128, HPC, E], F8, kind="ExternalInput")
    bqk_d = nc.dram_tensor("bqk", [128, 4 * HPC], F32, kind="ExternalInput")
    bv_d = nc.dram_tensor("bv_rep", [128, FH], BF16, kind="ExternalInput")
    cos_d = nc.dram_tensor("cos_t", [128, S], BF16, kind="ExternalInput")
    sin_d = nc.dram_tensor("sin_t", [128, S], BF16, kind="ExternalInput")
    cmask_d = nc.dram_tensor("cmask", [128, 4 * SB], BF16,
                             kind="ExternalInput")
    out_d = nc.dram_tensor("out", [S, E], BF16, kind="ExternalOutput")

    inv_sqrt_d = float(1.0 / np.sqrt(D))

    with TileContext(nc) as tc:
        with (
            tc.tile_pool(name="psum", bufs=2, space="PSUM") as psp,
            tc.tile_pool(name="cst", bufs=1) as cst,
            tc.tile_pool(name="wp", bufs=1) as wp,
            tc.tile_pool(name="kv", bufs=1) as kvp,
            tc.tile_pool(name="xp", bufs=2) as xp,
            tc.tile_pool(name="st", bufs=2) as st1,
            tc.tile_pool(name="aop", bufs=2) as aop,
            tc.tile_pool(name="osp", bufs=3) as osp,
        ):
            cm_t = cst.tile([128, 4 * SB], BF16, tag="cm")
            bqk_t = cst.tile([128, 4 * HPC], F32, tag="bqk")
            bv_t = cst.tile([128, FH], BF16, tag="bv")
            cos_t = cst.tile([128, S], BF16, tag="cos")
            sin_t = cst.tile([128, S], BF16, tag="sin")
            nc.sync.dma_start(out=cm_t[:], in_=cmask_d[:])
            nc.sync.dma_start(out=bqk_t[:], in_=bqk_d[:])
            nc.sync.dma_start(out=bv_t[:], in_=bv_d[:])

            def load_x(sb):
                ssl = slice(sb * SB, (sb + 1) * SB)
                xh = xp.tile([128, ECH, SB], F8, tag="xh", name="xh")
                xl = xp.tile([128, ECH, SB], F8, tag="xl", name="xl")
                nc.sync.dma_start(out=xh[:], in_=xhi_d[:, :, ssl])
                nc.sync.dma_start(out=xl[:], in_=xlo_d[:, :, ssl])
                return xh, xl

            # startup order: the first q chain needs x_hi + wq_hi first,
            # then x_lo + wq_lo (products 2/3), then rope tables.
            ssl0 = slice(0, SB)
            xh0 = xp.tile([128, ECH, SB], F8, tag="xh", name="xh0")
            xl0 = xp.tile([128, ECH, SB], F8, tag="xl", name="xl0")
            nc.sync.dma_start(out=xh0[:], in_=xhi_d[:, :, ssl0])
            wt = {}
            for nm in ("wq", "wk", "wv"):
                for hl in ("hi", "lo"):
                    wt[nm + hl] = wp.tile([128, ECH, FH], F8, tag=nm + hl,
                                          name=nm + hl)
            nc.sync.dma_start(out=wt["wqhi"][:], in_=w_d["wqhi"][:])
            nc.sync.dma_start(out=xl0[:], in_=xlo_d[:, :, ssl0])
            nc.sync.dma_start(out=wt["wqlo"][:], in_=w_d["wqlo"][:])
            xs0 = (xh0, xl0)
            nc.sync.dma_start(out=cos_t[:], in_=cos_d[:])
            nc.sync.dma_start(out=sin_t[:], in_=sin_d[:])
            for nm in ("wk", "wv"):
                for hl in ("hi", "lo"):
                    nc.sync.dma_start(out=wt[nm + hl][:], in_=w_d[nm + hl][:])
            wo_hi = wp.tile([128, HPC, E], F8, tag="wohi")
            wo_lo = wp.tile([128, HPC, E], F8, tag="wolo")
            nc.sync.dma_start(out=wo_hi[:], in_=wohi_d[:])
            nc.sync.dma_start(out=wo_lo[:], in_=wolo_d[:])

            # persistent k (per head, [d, S]) and v ([s128, (chunk, h, d)])
            kh = [kvp.tile([128, S], BF16, tag=f"kh{h}", name=f"kh{h}")
                  for h in range(HPC)]
            vh = kvp.tile([128, NST, FH], BF16, tag="vh")

            # PE warm-up: lifts the clock gate while initial DMAs fill.
            pwarm = psp.tile([128, SB], F32, tag="po", bufs=2, name="pwarm")
            for i in range(24):
                nc.tensor.matmul(pwarm[:], cm_t[:, 0:128], cm_t[:, 0:SB],
                                 start=(i == 0), stop=(i == 23))

            def emit_qk(sb, h, xs):
                """Project+rope q and k for head h of s-block sb."""
                xh, xl = xs
                ssl = slice(sb * SB, (sb + 1) * SB)
                fsl = slice(h * 128, (h + 1) * 128)
                qtile = None
                for kind in ("q", "k"):
                    whi = wt[("wq" if kind == "q" else "wk") + "hi"]
                    wlo = wt[("wq" if kind == "q" else "wk") + "lo"]
                    ps = psp.tile([128, SB], F32, tag="ps1", bufs=3,
                                  name="psqk")
                    for j in range(NPAIR):
                        jp = slice(2 * j, 2 * j + 2)
                        nc.tensor.matmul(ps[:], whi[:, jp, fsl], xh[:, jp, :],
                                         start=(j == 0), stop=False,
                                         perf_mode=DR)
                    for j in range(NPAIR):
                        jp = slice(2 * j, 2 * j + 2)
                        nc.tensor.matmul(ps[:], whi[:, jp, fsl], xl[:, jp, :],
                                         start=False, stop=False,
                                         perf_mode=DR)
                    for j in range(NPAIR):
                        jp = slice(2 * j, 2 * j + 2)
                        nc.tensor.matmul(ps[:], wlo[:, jp, fsl], xh[:, jp, :],
                                         start=False, stop=(j == NPAIR - 1),
                                         perf_mode=DR)
                    bofs = 0 if kind == "q" else 2 * HPC
                    bias = bqk_t[:, bofs + h:bofs + h + 1]
                    bias_sw = bqk_t[:, bofs + HPC + h:bofs + HPC + h + 1]
                    t1 = st1.tile([128, SB], BF16, tag="t1", bufs=2)
                    nc.vector.scalar_tensor_tensor(
                        out=t1[:], in0=ps[:], scalar=bias,
                        in1=cos_t[:, ssl], op0=AX.add, op1=AX.mult)
                    t2 = st1.tile([128, SB], BF16, tag="t2", bufs=2)
                    nc.vector.scalar_tensor_tensor(
                        out=t2[0:64, :], in0=ps[64:128, :],
                        scalar=bias_sw[0:64], in1=sin_t[0:64, ssl],
                        op0=AX.add, op1=AX.mult)
                    nc.vector.scalar_tensor_tensor(
                        out=t2[64:128, :], in0=ps[0:64, :],
                        scalar=bias_sw[64:128], in1=sin_t[64:128, ssl],
                        op0=AX.add, op1=AX.mult)
                    if kind == "q":
                        dst = st1.tile([128, SB], BF16, tag="qh", bufs=8,
                                       name="qh")
                        qtile = dst
                        dview = dst[:]
                    else:
                        dview = kh[h][:, ssl]
                    nc.gpsimd.tensor_add(dview, t1[:], t2[:])
                return qtile

            def emit_v(sb, xs):
                xh, xl = xs
                for ssub in range(SB // 128):
                    scol = slice(ssub * 128, (ssub + 1) * 128)
                    ps = psp.tile([128, FH], F32, tag="ps1", bufs=3,
                                  name="psv")
                    for j in range(NPAIR):
                        jp = slice(2 * j, 2 * j + 2)
                        nc.tensor.matmul(ps[:], xh[:, jp, scol],
                                         wt["wvhi"][:, jp, :],
                                         start=(j == 0), stop=False,
                                         perf_mode=DR)
                    for j in range(NPAIR):
                        jp = slice(2 * j, 2 * j + 2)
                        nc.tensor.matmul(ps[:], xl[:, jp, scol],
                                         wt["wvhi"][:, jp, :],
                                         start=False, stop=False,
                                         perf_mode=DR)
                    for j in range(NPAIR):
                        jp = slice(2 * j, 2 * j + 2)
                        nc.tensor.matmul(ps[:], xh[:, jp, scol],
                                         wt["wvlo"][:, jp, :],
                                         start=False, stop=(j == NPAIR - 1),
                                         perf_mode=DR)
                    # vh = ps/PRJ + bv  (bf16)
                    nc.vector.scalar_tensor_tensor(
                        out=vh[:, sb * 4 + ssub, :], in0=ps[:],
                        scalar=float(1.0 / PRJ), in1=bv_t[:],
                        op0=AX.mult, op1=AX.add)

            def emit_attn(sb, h, qtile, aohi, aolo):
                """Causal attention q-tile sb for head h (bf16)."""
                nk = (sb + 1) * 4
                po = psp.tile([128, SB], F32, tag="po", bufs=2, name="po")
                dacc = st1.tile([128, SB], F32, tag="dacc", bufs=2)
                pexps = []      # (ki, pexp, q0)
                pending = []
                for ki in range(nk):
                    j = ki - sb * 4
                    # diagonal chunks start at their pair's q0 so pair ops
                    # see fully-masked zeros in the extension
                    q0 = 128 * (j - (j % 2)) if j > 0 else 0
                    ksl = slice(ki * 128, (ki + 1) * 128)
                    pscore = psp.tile([128, SB], F32, tag="pscore",
                                      bufs=3, name="pscore")
                    nc.tensor.matmul(pscore[:, q0:SB], kh[h][:, ksl],
                                     qtile[:, q0:SB], start=True, stop=True)
                    pexp = st1.tile([128, SB], BF16, tag="pexp", bufs=8)
                    nc.scalar.activation(pexp[:, q0:SB], pscore[:, q0:SB],
                                         ACTF.Exp, scale=inv_sqrt_d)
                    if j >= 0:
                        nc.vector.tensor_mul(
                            pexp[:, q0:SB], pexp[:, q0:SB],
                            cm_t[:, j * SB + q0:(j + 1) * SB])
                    pexps.append((ki, pexp, q0))
                    if ki % 2 == 1:
                        _, pa, pq0 = pexps[ki - 1]
                        if ki == 1:
                            nc.vector.tensor_add(dacc[:, pq0:SB],
                                                 pa[:, pq0:SB],
                                                 pexp[:, pq0:SB])
                        else:
                            sp = st1.tile([128, SB], BF16, tag="spair",
                                          bufs=2)
                            nc.vector.tensor_add(sp[:, pq0:SB],
                                                 pa[:, pq0:SB],
                                                 pexp[:, pq0:SB])
                            nc.vector.tensor_add(dacc[:, pq0:SB],
                                                 dacc[:, pq0:SB],
                                                 sp[:, pq0:SB])
                    pending.append((ki, pexp, q0))
                    if len(pending) > 3:
                        k0, px, pq = pending.pop(0)
                        nc.tensor.matmul(
                            po[:, pq:SB],
                            vh[:, k0, h * 128:(h + 1) * 128],
                            px[:, pq:SB], start=(k0 == 0), stop=False)
                while pending:
                    k0, px, pq = pending.pop(0)
                    last = not pending
                    nc.tensor.matmul(po[:, pq:SB],
                                     vh[:, k0, h * 128:(h + 1) * 128],
                                     px[:, pq:SB], start=(k0 == 0), stop=last)
                dred = st1.tile([128, SB], F32, tag="dred", bufs=2)
                nc.gpsimd.partition_all_reduce(
                    out_ap=dred[:], in_ap=dacc[:], channels=128,
                    reduce_op=__import__("concourse.bass_isa",
                                         fromlist=["ReduceOp"]).ReduceOp.add)
                rec = st1.tile([128, SB], F32, tag="rec", bufs=2)
                nc.vector.reciprocal(rec[:], dred[:])
                t = st1.tile([128, SB], BF16, tag="taot", bufs=2)
                nc.vector.scalar_tensor_tensor(
                    out=t[:], in0=po[:], scalar=SAO, in1=rec[:],
                    op0=AX.mult, op1=AX.mult)
                nc.scalar.copy(aohi[:, h, :], t[:])
                nc.vector.tensor_sub(aolo[:, h, :], t[:], aohi[:, h, :])

            def emit_outproj_piece(sb, sti, aohi, aolo):
                """Out-projection for one 128-row s-chunk of s-block sb.
                Head-pair-major chain order so the first half only depends
                on heads 0/1."""
                stsl = slice(sti * 128, (sti + 1) * 128)
                osb = osp.tile([128, E], BF16, tag="osb", name="osb")
                for gt in range(E // 512):
                    gsl = slice(gt * 512, (gt + 1) * 512)
                    psO = psp.tile([128, 512], F32, tag="ps1", bufs=3,
                                   name="psO")
                    n = 0
                    for j in range(HPC // 2):
                        jp = slice(2 * j, 2 * j + 2)
                        for lhs, rhs in ((aohi, wo_hi), (aolo, wo_hi),
                                         (aohi, wo_lo)):
                            nc.tensor.matmul(psO[:], lhs[:, jp, stsl],
                                             rhs[:, jp, gsl],
                                             start=(n == 0), stop=(n == 5),
                                             perf_mode=DR)
                            n += 1
                    nc.scalar.activation(osb[:, gsl], psO[:], ACTF.Copy,
                                         scale=float(1.0 / (SAO * SW)))
                row0 = sb * SB + sti * 128
                nc.sync.dma_start(out=out_d[row0:row0 + 128, :],
                                  in_=osb[:])

            # ---- stage loop: attn(sb) interleaved with proj(sb+1) and
            # out-projection pieces of sb-1 ----
            q_cur = [emit_qk(0, h, xs0) for h in range(HPC)]
            emit_v(0, xs0)
            prev = None     # (sb-1, aohi, aolo) with outproj still pending
            for sb in range(NSB):
                nxt = sb + 1
                if nxt < NSB:
                    xs_n = load_x(nxt)
                aohi = aop.tile([128, HPC, SB], F8, tag="aohi", name="aohi")
                aolo = aop.tile([128, HPC, SB], F8, tag="aolo", name="aolo")
                q_next = []
                for h in range(HPC):
                    emit_attn(sb, h, q_cur[h], aohi, aolo)
                    if nxt < NSB:
                        q_next.append(emit_qk(nxt, h, xs_n))
                    if prev is not None:
                        emit_outproj_piece(prev[0], h, prev[1], prev[2])
                if nxt < NSB:
                    emit_v(nxt, xs_n)
                prev = (sb, aohi, aolo)
                q_cur = q_next
            for sti in range(SB // 128):
                emit_outproj_piece(prev[0], sti, prev[1], prev[2])

    nc.compile()
    return nc


def _host_constants():
    """RoPE cos/sin tables (evens-first, pre-descaled) and causal masks."""
    i = np.arange(64, dtype=np.float64)
    freqs = np.power(10000.0, -2.0 * i / D)
    pos = np.arange(S, dtype=np.float64)
    ang = pos[None, :] * freqs[:, None]              # [64, S]
    cos = np.cos(ang)
    sin = np.sin(ang)
    cos_t = (np.concatenate([cos, cos], axis=0) / PRJ).astype(BFNP)
    sin_t = (np.concatenate([-sin, sin], axis=0) / PRJ).astype(BFNP)
    r = np.arange(128)[:, None]
    c = np.arange(SB)[None, :]
    masks = [(128 * j + r <= c).astype(np.float32) for j in range(4)]
    cmask = np.concatenate(masks, axis=1).astype(BFNP)
    return cos_t, sin_t, cmask


def _split8(t, s):
    hi = (s * t).astype(F8NP)
    lo = (s * t - hi.astype(np.float32)).astype(F8NP)
    return hi, lo


def _chunked(t, nch):
    """[nch*128, N] f8 -> [128, nch, N]"""
    n = t.shape[1]
    return np.ascontiguousarray(
        t.reshape(nch, 128, n).transpose(1, 0, 2))


def kernel(x, Wq, bq, Wk, bk, Wv, bv, Wo, bo):
    x = np.asarray(x, dtype=np.float32)
    Wq = np.asarray(Wq, dtype=np.float32)
    bq = np.asarray(bq, dtype=np.float32)
    Wk = np.asarray(Wk, dtype=np.float32)
    bk = np.asarray(bk, dtype=np.float32)
    Wv = np.asarray(Wv, dtype=np.float32)
    bv = np.asarray(bv, dtype=np.float32)
    Wo = np.asarray(Wo, dtype=np.float32)
    bo = np.asarray(bo, dtype=np.float32)

    if "nc" not in _CACHE:
        _CACHE["nc"] = _build_program()
        _CACHE["consts"] = _host_constants()
    nc = _CACHE["nc"]
    cos_t, sin_t, cmask = _CACHE["consts"]

    perm = np.concatenate([np.arange(0, D, 2), np.arange(1, D, 2)])
    sw64 = np.concatenate([np.arange(64, 128), np.arange(0, 64)])

    xsplit = []
    for b in range(B):
        xT = np.ascontiguousarray(x[b].T)
        xh, xl = _split8(xT, SX)
        xsplit.append((_chunked(xh, ECH), _chunked(xl, ECH)))

    in_maps = []
    for c in range(NCORES):
        b, hg = divmod(c, GROUPS)
        rows = slice(hg * FH, (hg + 1) * FH)
        Wq_s = Wq[rows].reshape(HPC, D, E)[:, perm, :].reshape(FH, E)
        Wk_s = Wk[rows].reshape(HPC, D, E)[:, perm, :].reshape(FH, E)
        bq_s = bq[rows].reshape(HPC, D)[:, perm]     # [HPC, 128]
        bk_s = bk[rows].reshape(HPC, D)[:, perm]
        bqk_t = PRJ * np.concatenate(
            [bq_s, bq_s[:, sw64], bk_s, bk_s[:, sw64]],
            axis=0).T.astype(np.float32)
        bqk_t = np.ascontiguousarray(bqk_t)          # [128, 4*HPC]

        wqh, wql = _split8(np.ascontiguousarray(Wq_s.T), SW)
        wkh, wkl = _split8(np.ascontiguousarray(Wk_s.T), SW)
        wvh, wvl = _split8(np.ascontiguousarray(Wv[rows].T), SW)
        woh, wol = _split8(np.ascontiguousarray(Wo[:, rows].T), SW)

        in_maps.append({
            "xhi": xsplit[b][0],
            "xlo": xsplit[b][1],
            "wqhi": _chunked(wqh, ECH), "wqlo": _chunked(wql, ECH),
            "wkhi": _chunked(wkh, ECH), "wklo": _chunked(wkl, ECH),
            "wvhi": _chunked(wvh, ECH), "wvlo": _chunked(wvl, ECH),
            "wohi": _chunked(woh, HPC), "wolo": _chunked(wol, HPC),
            "bqk": bqk_t,
            "bv_rep": np.ascontiguousarray(
                np.broadcast_to(bv[rows], (128, FH))).astype(BFNP),
            "cos_t": cos_t,
            "sin_t": sin_t,
            "cmask": cmask,
        })

    res = run_bass_kernel_spmd(nc, in_maps, list(range(NCORES)))
    outs = [res.results[c]["out"] for c in range(NCORES)]

    result = np.empty((B, S, E), dtype=np.float32)
    for b in range(B):
        acc = outs[GROUPS * b].astype(np.float32)
        for g in range(1, GROUPS):
            acc = acc + outs[GROUPS * b + g].astype(np.float32)
        result[b] = acc + bo[None, :]
    return result
